# revision 1
# baseline (speedup 1.0000x reference)
"""Trainium2 Bass kernel for pre-norm MHA (nn_Attention_55009941128016).

Sharding: 8 cores = 4 batches x 2 head-groups (8 heads each); host sums the
two tensor-parallel partials per batch.

Structure: LN+transpose -> v-projection (SBUF-resident, bf16, ones column
for the softmax denominators) -> qk+rope per head pair -> transposed
attention (dots^T = k @ q^T, exp on ACT, denominators ride the av matmul) ->
out-projection (bf16) with b_out applied by the DVE evacuation.

vs the previous checkpoint: v no longer round-trips through DRAM; exp/v/
ohT/w_out are bf16 (rel err 2.8e-3 vs fp32 reference, gate 2e-2); the K=1
bias matmuls are gone; q0/rope evacuations run on DVE; xc/ident are f32r so
the PE transposes run at 1.5 cyc/row. Measured min-slope 569us vs 588us.
"""

import os
import sys

import numpy as np

for _p in ("/opt/trn_rl_repo", "/root/.axon_site/_ro/trn_rl_repo"):
    if os.path.isdir(_p) and _p not in sys.path:
        sys.path.append(_p)

B = 4
SEQ = 2048
DIM = 1024
HEADS = 16
DIM_HEAD = 64
N_CORES = 8
NH_C = 8          # heads per core
PAIRS = NH_C // 2
KC = DIM // 128   # 8 contraction chunks
EPS = 1e-5

_cache = {}


def _rope_tables():
    pos = np.arange(SEQ, dtype=np.float64)
    inv = 1.0 / (10000.0 ** (np.arange(0, DIM_HEAD // 2, dtype=np.float64) / (DIM_HEAD // 2)))
    ang = pos[:, None] * inv[None, :]                      # [n, 32]
    cos = np.repeat(np.cos(ang), 2, axis=-1)               # [n, 64]
    sin = np.repeat(np.sin(ang), 2, axis=-1)
    cosT = cos.T.astype(np.float32)                        # [64, n]
    sinT = sin.T.astype(np.float32)
    sgn = np.ones((DIM_HEAD, 1), np.float32)
    sgn[: DIM_HEAD // 2] = -1.0                            # rotate_half sign, folded into sin
    sinT = sinT * sgn
    cos2 = np.concatenate([cosT, cosT], axis=0).copy()     # [128, n] (2 heads per chunk)
    sin2 = np.concatenate([sinT, sinT], axis=0).copy()
    return cos2, sin2


def _perm_matrix():
    # shufq = P @ qT with shufq[d] = qT[(d+32)%64 within each 64-block]; lhsT = P.T
    P = np.zeros((128, 128), np.float32)
    for blk in range(2):
        for d in range(DIM_HEAD):
            P[blk * 64 + d, blk * 64 + (d + 32) % 64] = 1.0
    return P.T.copy()


def _build_program(has_lnb, reps=1):
    import concourse.tile as tile
    from concourse import bacc, mybir
    from contextlib import ExitStack

    f32 = mybir.dt.float32
    f32r = mybir.dt.float32r
    bf16 = mybir.dt.bfloat16
    Alu = mybir.AluOpType
    Act = mybir.ActivationFunctionType

    rf = f32r

    nc = bacc.Bacc("TRN2", target_bir_lowering=False, debug=False, num_devices=N_CORES)

    x_d = nc.dram_tensor("x", [SEQ, DIM], f32, kind="ExternalInput").ap()
    wq_d = nc.dram_tensor("wq", [128, KC, 512], rf, kind="ExternalInput").ap()
    wk_d = nc.dram_tensor("wk", [128, KC, 512], rf, kind="ExternalInput").ap()
    wv_d = nc.dram_tensor("wv", [128, KC, 512], rf, kind="ExternalInput").ap()
    wo_d = nc.dram_tensor("wo", [128, 4, DIM], bf16, kind="ExternalInput").ap()
    bq_d = nc.dram_tensor("bq", [128, 4], f32, kind="ExternalInput").ap()
    bk_d = nc.dram_tensor("bk", [128, 4], f32, kind="ExternalInput").ap()
    bv_d = nc.dram_tensor("bv", [64, NH_C], f32, kind="ExternalInput").ap()
    cos_d = nc.dram_tensor("cos2", [128, SEQ], f32, kind="ExternalInput").ap()
    sin_d = nc.dram_tensor("sin2", [128, SEQ], f32, kind="ExternalInput").ap()
    pm_d = nc.dram_tensor("pmatT", [128, 128], rf, kind="ExternalInput").ap()
    id_d = nc.dram_tensor("ident", [128, 128], rf, kind="ExternalInput").ap()
    bias_d = nc.dram_tensor("bias_bc", [128, DIM], f32, kind="ExternalInput").ap()
    out_d = nc.dram_tensor("out_p", [SEQ, DIM], f32, kind="ExternalOutput").ap()

    with tile.TileContext(nc) as tc, ExitStack() as top:
        persist = top.enter_context(tc.tile_pool(name="persist", bufs=1))

        ident = persist.tile([128, 128], rf, tag="ident")
        pmat = persist.tile([128, 128], rf, tag="pmat")
        bqc = persist.tile([128, 4], f32, tag="bqc")
        bkc = persist.tile([128, 4], f32, tag="bkc")
        bvc = persist.tile([64, NH_C], f32, tag="bvc")

        nc.sync.dma_start(ident[:], id_d[:])
        nc.sync.dma_start(pmat[:], pm_d[:])
        nc.sync.dma_start(bqc[:], bq_d[:])
        nc.sync.dma_start(bkc[:], bk_d[:])
        nc.sync.dma_start(bvc[:], bv_d[:])

        rep_ctx = tc.For_i(0, reps, 1) if reps > 1 else None
        if rep_ctx is not None:
            rep_ctx.__enter__()

        fwd = ExitStack()
        xnp = fwd.enter_context(tc.tile_pool(name="xnp", bufs=1))
        xnT = xnp.tile([128, KC, SEQ], rf, tag="xnT")              # 64KB/p

        # ---------------- Phase 1: LN + transpose -> xnT ----------------
        with ExitStack() as ph1:
            xio = ph1.enter_context(tc.tile_pool(name="xio", bufs=4))
            small = ph1.enter_context(tc.tile_pool(name="small", bufs=4))
            ps_t = ph1.enter_context(tc.tile_pool(name="ps_t", bufs=2, space="PSUM"))

            for t in range(16):
                xt = xio.tile([128, DIM], f32, tag="xt")
                nc.sync.dma_start(xt[:], x_d[t * 128:(t + 1) * 128, :])
                st6 = small.tile([128, 12], f32, tag="st6")
                nc.vector.bn_stats(st6[:, 0:6], xt[:, 0:512])
                nc.vector.bn_stats(st6[:, 6:12], xt[:, 512:1024])
                mv = small.tile([128, 2], f32, tag="mv")
                nc.vector.bn_aggr(mv[:], st6[:])
                veps = small.tile([128, 1], f32, tag="veps")
                nc.vector.tensor_scalar_add(veps[:], mv[:, 1:2], EPS)
                sig = small.tile([128, 1], f32, tag="sig")
                nc.scalar.activation(sig[:], veps[:], Act.Sqrt)
                rst = small.tile([128, 1], f32, tag="rst")
                nc.vector.reciprocal(rst[:], sig[:])
                xc = xio.tile([128, DIM], rf, tag="xc")
                # split the big normalize op between DVE and GPSIMD
                eng = nc.vector if (t % 2 == 0) else nc.gpsimd
                eng.tensor_scalar(
                    xc[:], xt[:], mv[:, 0:1], rst[:], Alu.subtract, Alu.mult
                )
                for g2 in range(2):
                    pst = ps_t.tile([128, 512], rf, tag="pst")
                    for k in range(4):
                        c = g2 * 4 + k
                        nc.tensor.transpose(
                            pst[:, k * 128:(k + 1) * 128],
                            xc[:, c * 128:(c + 1) * 128],
                            ident[:],
                        )
                    nc.scalar.activation(
                        xnT[:, g2 * 4:(g2 + 1) * 4, t * 128:(t + 1) * 128],
                        pst[:].rearrange("p (c n) -> p c n", c=4),
                        Act.Copy,
                    )

        # v lives in SBUF (bf16) until the end of attention
        vpool_scope = ExitStack()
        vsp = vpool_scope.enter_context(tc.tile_pool(name="vsp", bufs=1))
        v_sb = vsp.tile([128, 16, NH_C, 65], bf16, tag="v_sb")     # 16.6KB/p
        nc.gpsimd.memset(v_sb[:, :, :, 64:65], 1.0)                # ones column
        # -> av psum row 64 = softmax denominator

        # ---------------- Phase 2b: v (token-major) -> SBUF ----------------
        with ExitStack() as ph2b:
            wvp = ph2b.enter_context(tc.tile_pool(name="wvp", bufs=1))
            ps_mm2 = ph2b.enter_context(tc.tile_pool(name="ps_mm2", bufs=2, space="PSUM"))

            wv_sb = wvp.tile([128, KC, 512], rf, tag="wv")
            nc.sync.dma_start(wv_sb[:], wv_d[:])
            for m in range(16):
                ps = ps_mm2.tile([128, 512], f32, tag="qkv2")
                for kc in range(KC):
                    nc.tensor.matmul(
                        ps[:], xnT[:, kc, m * 128:(m + 1) * 128], wv_sb[:, kc, :],
                        start=(kc == 0), stop=(kc == KC - 1),
                    )
                nc.scalar.activation(
                    v_sb[:, m, :, 0:64],
                    ps[:].rearrange("p (h d) -> p h d", h=NH_C),
                    Act.Copy,
                )

        # ---------------- sequential qk+rope, then attention ----------------
        oht_scope = ExitStack()
        oht = oht_scope.enter_context(tc.tile_pool(name="oht", bufs=1))
        ohT = oht.tile([128, 4, SEQ], bf16, tag="ohT")             # 16KB/p

        qkt_scope = ExitStack()
        qkt = qkt_scope.enter_context(tc.tile_pool(name="qkt", bufs=1))
        qT = qkt.tile([128, 4, SEQ], rf, tag="qT")                 # 32KB/p
        kT = qkt.tile([128, 4, SEQ], rf, tag="kT")                 # 32KB/p

        qk_scope = ExitStack()
        tabs = qk_scope.enter_context(tc.tile_pool(name="tabs", bufs=1))
        wpool = qk_scope.enter_context(tc.tile_pool(name="wpool", bufs=2))
        rtmp = qk_scope.enter_context(tc.tile_pool(name="rtmp", bufs=2))
        ps_qk = qk_scope.enter_context(tc.tile_pool(name="ps_qk", bufs=3, space="PSUM"))
        ps_pm = qk_scope.enter_context(tc.tile_pool(name="ps_pm", bufs=2, space="PSUM"))

        cos_sb = tabs.tile([128, SEQ], f32, tag="cos")
        sin_sb = tabs.tile([128, SEQ], f32, tag="sin")
        nc.sync.dma_start(cos_sb[:], cos_d[:])
        nc.sync.dma_start(sin_sb[:], sin_d[:])

        def qk_rope(p):
            for which, w_dram, dest, bcol in (("q", wq_d, qT, bqc), ("k", wk_d, kT, bkc)):
                wt = wpool.tile([128, KC, 128], rf, tag="wqk", name=f"w{which}{p}")
                nc.sync.dma_start(wt[:], w_dram[:, :, p * 128:(p + 1) * 128])
                for i in range(4):
                    isl = slice(i * 512, (i + 1) * 512)
                    ps = ps_qk.tile([128, 512], f32, tag="qkps")
                    for kc in range(KC):
                        nc.tensor.matmul(
                            ps[:], wt[:, kc, :], xnT[:, kc, isl],
                            start=(kc == 0), stop=(kc == KC - 1),
                        )
                    q0 = rtmp.tile([128, 512], rf, tag="q0")
                    nc.vector.tensor_copy(q0[:], ps[:])
                    ps2 = ps_pm.tile([128, 512], f32, tag="pmm")
                    nc.tensor.matmul(ps2[:], pmat[:], q0[:], start=True, stop=True)
                    t1 = rtmp.tile([128, 512], f32, tag="t1")
                    nc.gpsimd.tensor_tensor(t1[:], q0[:], cos_sb[:, isl], Alu.mult)
                    t2 = rtmp.tile([128, 512], f32, tag="t2")
                    nc.vector.tensor_tensor(t2[:], ps2[:], sin_sb[:, isl], Alu.mult)
                    nc.vector.scalar_tensor_tensor(
                        dest[:, p, isl], t1[:], bcol[:, p:p + 1], t2[:],
                        Alu.add, Alu.add,
                    )

        for p in range(PAIRS):
            qk_rope(p)
        qk_scope.close()    # frees tables, rope tmps, all qk PSUM banks

        attn_scope = ExitStack()
        expool = attn_scope.enter_context(tc.tile_pool(name="expool", bufs=3))
        dvt = attn_scope.enter_context(tc.tile_pool(name="dvt", bufs=1))
        ps_dots = attn_scope.enter_context(tc.tile_pool(name="ps_dots", bufs=2, space="PSUM"))
        ps_av = attn_scope.enter_context(tc.tile_pool(name="ps_av", bufs=4, space="PSUM"))

        def attn(p):
            hA, hB = 2 * p, 2 * p + 1
            for i in range(4):
                isl = slice(i * 512, (i + 1) * 512)
                avA = ps_av.tile([65, 512], f32, tag="av", name=f"avA_{p}_{i}")
                avB = ps_av.tile([65, 512], f32, tag="av", name=f"avB_{p}_{i}")
                for jt in range(16):
                    jsl = slice(jt * 128, (jt + 1) * 128)
                    dts = ps_dots.tile([128, 1024], f32, tag="dots")
                    nc.tensor.matmul(
                        dts[:, 0:512], kT[0:64, p, jsl], qT[0:64, p, isl],
                        start=True, stop=True, tile_position=(0, 0),
                    )
                    nc.tensor.matmul(
                        dts[:, 512:1024], kT[64:128, p, jsl], qT[64:128, p, isl],
                        start=True, stop=True, tile_position=(64, 0),
                    )
                    ex = expool.tile([128, 1024], bf16, tag="exp")
                    nc.scalar.activation(ex[:], dts[:], Act.Exp)
                    nc.tensor.matmul(
                        avA[:], v_sb[:, jt, hA, :], ex[:, 0:512],
                        start=(jt == 0), stop=(jt == 15),
                    )
                    nc.tensor.matmul(
                        avB[:], v_sb[:, jt, hB, :], ex[:, 512:1024],
                        start=(jt == 0), stop=(jt == 15),
                    )
                # denominators (psum row 64): copy to SBUF, DMA to partition 0,
                # reciprocal on SBUF, broadcast, divide (baseline-proven path)
                stg = dvt.tile([65, 1024], f32, tag="stg")
                nc.vector.tensor_copy(stg[64:65, 0:512], avA[64:65, :])
                nc.vector.tensor_copy(stg[64:65, 512:1024], avB[64:65, :])
                rin = dvt.tile([1, 1024], f32, tag="rin")
                nc.sync.dma_start(rin[0:1, :], stg[64:65, :])
                rout = dvt.tile([1, 1024], f32, tag="rout")
                rscr = dvt.tile([1, 1024], f32, tag="rscr")
                nc.vector.reciprocal_approx_accurate(rout[:], rin[:], rscr[:])
                bcA = dvt.tile([64, 512], f32, tag="bcA")
                bcB = dvt.tile([64, 512], f32, tag="bcB")
                nc.gpsimd.partition_broadcast(bcA[:], rout[0:1, 0:512])
                nc.gpsimd.partition_broadcast(bcB[:], rout[0:1, 512:1024])
                nc.vector.tensor_tensor(
                    ohT[0:64, p, isl], avA[0:64, :], bcA[:], Alu.mult
                )
                tmpB = dvt.tile([64, 512], bf16, tag="tmpB")
                nc.vector.tensor_tensor(tmpB[:], avB[0:64, :], bcB[:], Alu.mult)
                if has_lnb:
                    nc.vector.tensor_scalar_add(
                        ohT[0:64, p, isl], ohT[0:64, p, isl], bvc[:, hA:hA + 1]
                    )
                    nc.vector.tensor_scalar_add(
                        tmpB[:], tmpB[:], bvc[:, hB:hB + 1]
                    )
                nc.sync.dma_start(ohT[64:128, p, isl], tmpB[:])

        for p in range(PAIRS):
            attn(p)

        attn_scope.close()
        qkt_scope.close()   # frees qT/kT

        # ---------------- Phase 4: out projection + bias ----------------
        with ExitStack() as ph4:
            wo_sb_p = ph4.enter_context(tc.tile_pool(name="wo", bufs=1))
            ops = ph4.enter_context(tc.tile_pool(name="ops", bufs=3))
            ps_o = ph4.enter_context(tc.tile_pool(name="ps_o", bufs=2, space="PSUM"))

            wo_sb = wo_sb_p.tile([128, 4, DIM], bf16, tag="wo")
            nc.sync.dma_start(wo_sb[:], wo_d[:])
            bias_bc = wo_sb_p.tile([128, DIM], f32, tag="bias_bc")
            nc.sync.dma_start(bias_bc[:], bias_d[:])

            for it in range(16):
                tsl = slice(it * 128, (it + 1) * 128)
                ot = ops.tile([128, DIM], f32, tag="ot")
                for oc in range(2):
                    osl = slice(oc * 512, (oc + 1) * 512)
                    ps = ps_o.tile([128, 512], f32, tag="op")
                    for c in range(4):
                        nc.tensor.matmul(
                            ps[:], ohT[:, c, tsl], wo_sb[:, c, osl],
                            start=(c == 0), stop=(c == 3),
                        )
                    nc.vector.tensor_tensor(ot[:, osl], ps[:], bias_bc[:, osl], Alu.add)
                nc.sync.dma_start(out_d[tsl, :], ot[:])

        oht_scope.close()
        vpool_scope.close()
        fwd.close()         # frees xnT
        if rep_ctx is not None:
            rep_ctx.__exit__(None, None, None)

    nc.compile()
    return nc


def _prep_inputs(x, ln_g, ln_b, w_qkv, w_out, b_out):
    import ml_dtypes

    x = np.asarray(x, np.float32)
    ln_g = np.asarray(ln_g, np.float32)
    ln_b = np.asarray(ln_b, np.float32)
    w_qkv = np.asarray(w_qkv, np.float32)
    w_out = np.asarray(w_out, np.float32)
    b_out = np.asarray(b_out, np.float32)

    cos2, sin2 = _rope_tables()
    pmatT = _perm_matrix()
    ident = np.eye(128, dtype=np.float32)
    has_lnb = bool(np.any(ln_b != 0.0))

    def fold(mat):  # [1024, 512] -> [128, 8, 512]
        return np.ascontiguousarray(mat.reshape(KC, 128, 512).transpose(1, 0, 2))

    in_maps = []
    for c in range(N_CORES):
        b, g = c // 2, c % 2
        qs = slice(g * 512, (g + 1) * 512)
        ks = slice(DIM + g * 512, DIM + (g + 1) * 512)
        vs = slice(2 * DIM + g * 512, 2 * DIM + (g + 1) * 512)
        wq = fold(ln_g[:, None] * w_qkv[:, qs]) / 8.0
        wk = fold(ln_g[:, None] * w_qkv[:, ks])
        wv = fold(ln_g[:, None] * w_qkv[:, vs])
        bq = (ln_b @ w_qkv[:, qs]) / 8.0
        bk = ln_b @ w_qkv[:, ks]
        bv = ln_b @ w_qkv[:, vs]
        wo = np.ascontiguousarray(
            w_out[g * 512:(g + 1) * 512, :].reshape(4, 128, DIM).transpose(1, 0, 2)
        ).astype(ml_dtypes.bfloat16)
        bias_full = (b_out if g == 0 else np.zeros_like(b_out)).reshape(1, DIM)
        in_maps.append({
            "x": np.ascontiguousarray(x[b]),
            "wq": wq.astype(np.float32),
            "wk": wk.astype(np.float32),
            "wv": wv.astype(np.float32),
            "wo": wo,
            "bq": np.ascontiguousarray(bq.reshape(4, 128).T).astype(np.float32),
            "bk": np.ascontiguousarray(bk.reshape(4, 128).T).astype(np.float32),
            "bv": np.ascontiguousarray(bv.reshape(NH_C, 64).T).astype(np.float32),
            "cos2": cos2,
            "sin2": sin2,
            "pmatT": pmatT,
            "ident": ident,
            "bias_bc": np.ascontiguousarray(np.broadcast_to(bias_full, (128, DIM))).astype(np.float32),
        })
    return in_maps, has_lnb


def _get_program(has_lnb, reps=1):
    key = ("prog", has_lnb, reps)
    if key not in _cache:
        _cache[key] = _build_program(has_lnb, reps)
    return _cache[key]


def kernel(**inputs):
    from concourse.bass_utils import run_bass_kernel_spmd

    in_maps, has_lnb = _prep_inputs(**inputs)
    nc = _get_program(has_lnb)
    res = run_bass_kernel_spmd(nc, in_maps, list(range(N_CORES))).results
    out = np.empty((B, SEQ, DIM), np.float32)
    for b in range(B):
        out[b] = res[2 * b]["out_p"] + res[2 * b + 1]["out_p"]
    return out

